# revision 8
# baseline (speedup 1.0000x reference)
"""Trainium2 Bass kernel for nn_Att_LSTM_67989332296335.

Math note: the reference's attention softmax is over a singleton axis, so
A == 1 identically and G[t] = sum_j hs[j] for every t — the whole (S,S,B)
distance tensor is dead code.  The live computation is: embedding gather ->
1024-step LSTM -> hsum -> Z = tanh(G @ Wg1.T + hs @ Wg2.T + b_g) ->
tag = Z @ W_tag.T + b_tag -> log_softmax over the batch axis.

Distribution: data-parallel over batch B=32 across 8 cores (B_local=4),
per the sharding hint.  The LSTM recurrence runs per-core with a
column-tiled TensorE schedule: col group g owns hidden-unit chunk
U_g = [128g, 128g+128) x all 4 gates; weights stream as the moving operand
(7 K-chunks of N=512 per group per step: 2 emb + 1 bias-row + 4 W_hh) while
the tiny h.T / emb.T slices are stationary.  h -> h.T each step via 16
small col-tiled identity matmuls.  The batch log-softmax needs one 8-core
AllReduce of the (20, S) exp-sums.
"""
import sys
sys.path.insert(0, "/opt/trn_rl_repo")

import numpy as np
import ml_dtypes

import concourse.bass as bass
import concourse.tile as tile
from concourse import bacc, mybir

F32 = mybir.dt.float32
BF16 = mybir.dt.bfloat16
BF16_NP = ml_dtypes.bfloat16

S, B, E, H, AH, V, T = 1024, 32, 256, 512, 256, 32000, 20
NCORES = 8
BL = B // NCORES  # 4

_graph_cache = {}


def build_graph(steps=S, debug=False):
    nc = bacc.Bacc(None, target_bir_lowering=False, debug=debug)
    sb = steps * BL
    tpn = min(128, steps)                 # timesteps per P3/P4 N-tile
    nw = tpn * BL                         # N width (cols) per tile
    ntb = steps // tpn                    # number of N-tiles
    tw = min(128, sb)                     # output-transpose tile width
    ntr = sb // tw

    def P(name, shape, dt):
        return nc.dram_tensor(name, list(shape), dt, kind="ExternalInput")

    e0_d = P("e0", (128, sb + 32), BF16)
    e1_d = P("e1", (128, sb), BF16)
    ones_d = P("ones", (1, sb), BF16)
    wih2_d = P("wih2", (1, 2048), BF16)
    ones512_d = P("ones512", (1, 512), BF16)
    btag_d = P("btag", (1, 20), BF16)
    bgt_d = P("bgt", (1, 256), F32)
    wih0_d = P("wih0", (128, 2048), BF16)
    wih1_d = P("wih1", (128, 2048), BF16)
    whh_d = P("whh", (512, 2048), F32)
    i4s_d = P("i4s", (128, 4), F32)
    sm32_d = P("sm32", (128, 256), F32)
    wg1t_d = P("wg1t", (512, 256), F32)
    wg2t_d = P("wg2t", (512, 256), BF16)
    indrep_d = P("indrep", (4, 512), F32)
    wtagt_d = P("wtagt", (256, 20), BF16)
    i20_d = P("i20", (20, 20), F32)
    out_d = nc.dram_tensor("out", [sb, T], F32, kind="ExternalOutput")

    SIG = mybir.ActivationFunctionType.Sigmoid
    TANH = mybir.ActivationFunctionType.Tanh

    def strips(w, g):
        """Moving-operand AP: cols {512*gate + 128*g + [0:128)} of a (p, 2048) tile."""
        return w[:].rearrange("p (G C u) -> p G C u", G=4, C=4)[:, :, g, :]

    with tile.TileContext(nc) as tc:
        with tc.tile_pool(name="persist", bufs=1) as pp, \
             tc.tile_pool(name="dram", bufs=1, space="DRAM") as dp:
            e0 = pp.tile([128, sb + 32], BF16, tag="e0")
            e1 = pp.tile([128, sb], BF16, tag="e1")
            ones = pp.tile([1, sb], BF16, tag="ones")
            wih2 = pp.tile([1, 2048], BF16, tag="wih2")
            ones512 = pp.tile([1, 512], BF16, tag="ones512")
            btag = pp.tile([1, 20], BF16, tag="btag")
            bgt = pp.tile([1, 256], F32, tag="bgt")
            wih0 = pp.tile([128, 2048], BF16, tag="wih0")
            wih1 = pp.tile([128, 2048], BF16, tag="wih1")
            whh = [pp.tile([128, 2048], F32, name=f"whh{k}", tag=f"whh{k}") for k in range(4)]
            i4s = pp.tile([128, 4], F32, tag="i4s")
            sm32 = pp.tile([128, 256], F32, tag="sm32")
            wg1t = [pp.tile([128, 256], F32, name=f"wg1t{k}", tag=f"wg1t{k}") for k in range(4)]
            wg2t = [pp.tile([128, 256], BF16, name=f"wg2t{k}", tag=f"wg2t{k}") for k in range(4)]
            indrep = pp.tile([4, 512], F32, tag="indrep")
            wtagt = [pp.tile([128, 20], BF16, name=f"wtagt{k}", tag=f"wtagt{k}") for k in range(2)]
            i20 = pp.tile([20, 20], F32, tag="i20")

            hT = pp.tile([128, 16], F32, tag="hT")
            hsT = pp.tile([128, steps * 16], BF16, tag="hsT")
            cst = pp.tile([128, 128], F32, tag="cst")
            sif = pp.tile([128, 256], F32, tag="sif")
            tg = pp.tile([128, 128], F32, tag="tg")
            so = pp.tile([128, 128], F32, tag="so")
            tcl = pp.tile([128, 128], F32, tag="tcl")
            m1 = pp.tile([128, 128], F32, tag="m1")
            m2 = pp.tile([128, 128], F32, tag="m2")
            hn2 = pp.tile([4, 512], F32, tag="hn2")

            for t_, src in [(e0, e0_d), (e1, e1_d), (ones, ones_d),
                            (wih2, wih2_d), (ones512, ones512_d),
                            (btag, btag_d), (bgt, bgt_d),
                            (wih0, wih0_d), (wih1, wih1_d), (i4s, i4s_d),
                            (sm32, sm32_d), (indrep, indrep_d), (i20, i20_d)]:
                nc.sync.dma_start(t_[:], src[:])
            for k in range(4):
                nc.sync.dma_start(whh[k][:], whh_d[128 * k:128 * (k + 1), :])
                nc.sync.dma_start(wg1t[k][:], wg1t_d[128 * k:128 * (k + 1), :])
                nc.sync.dma_start(wg2t[k][:], wg2t_d[128 * k:128 * (k + 1), :])
            for k in range(2):
                nc.sync.dma_start(wtagt[k][:], wtagt_d[128 * k:128 * (k + 1), :])
            nc.vector.memset(hT[:], 0.0)
            nc.vector.memset(cst[:], 0.0)
            nc.vector.memset(hn2[:], 0.0)

            # ---- Phase 1: LSTM recurrence (fully unrolled) ----
            with tc.tile_pool(name="pz", bufs=2, space="PSUM") as pzp, \
                 tc.tile_pool(name="pt", bufs=2, space="PSUM") as ptp:
                for t in range(steps):
                    pz = pzp.tile([128, 512], F32, tag="pz")
                    for g in range(4):
                        o = pz[32 * g:32 * g + 4, 0:512]
                        tp = (0, 32 * g)
                        # M=32 start chunk defines the whole partition group
                        nc.tensor.matmul(pz[32 * g:32 * g + 32, 0:512],
                                         e0[:, 4 * t:4 * t + 32],
                                         strips(wih0, g),
                                         start=True, stop=False, tile_position=tp,
                                         skip_group_check=True)
                        nc.tensor.matmul(o, e1[:, 4 * t:4 * t + 4],
                                         strips(wih1, g),
                                         start=False, stop=False, tile_position=tp,
                                         skip_group_check=True)
                        nc.tensor.matmul(o, ones[0:1, 4 * t:4 * t + 4],
                                         strips(wih2, g),
                                         start=False, stop=False, tile_position=tp,
                                         skip_group_check=True)
                        for k in range(4):
                            nc.tensor.matmul(o, hT[:, 4 * k:4 * k + 4],
                                             strips(whh[k], g),
                                             start=False, stop=(k == 3),
                                             tile_position=tp,
                                             skip_group_check=True)
                    # gates: rows {32g+b} meaningful, full-width ops
                    nc.scalar.activation(sif[:], pz[:, 0:256], SIG)
                    nc.scalar.activation(tg[:], pz[:, 256:384], TANH)
                    nc.scalar.activation(so[:], pz[:, 384:512], SIG)
                    nc.vector.tensor_mul(m1[:], sif[:, 128:256], cst[:])
                    nc.vector.tensor_mul(m2[:], sif[:, 0:128], tg[:])
                    nc.vector.tensor_add(cst[:], m1[:], m2[:])
                    nc.scalar.activation(tcl[:], cst[:], TANH)
                    # h to batch-major (4, 512) at partition base 0
                    for g in range(4):
                        nc.vector.tensor_mul(hn2[0:4, 128 * g:128 * (g + 1)],
                                             tcl[32 * g:32 * g + 4, 0:128],
                                             so[32 * g:32 * g + 4, 0:128])
                    # h -> h.T via 16 tiny col-tiled identity matmuls
                    pt = ptp.tile([128, 16], F32, tag="pt")
                    for g in range(4):
                        for j in range(4):
                            nc.tensor.matmul(
                                pt[32 * j:32 * j + 32, 4 * g:4 * g + 4],
                                hn2[0:4, 128 * g + 32 * j:128 * g + 32 * j + 32],
                                i4s[0:4, 0:4],
                                start=True, stop=True,
                                tile_position=(0, 32 * j))
                    nc.scalar.copy(hT[:], pt[:])
                    nc.scalar.copy(hsT[:, 16 * t:16 * (t + 1)], pt[:])

            # ---- Phase 2: hsum and gGb ----
            hsum = pp.tile([128, 16], F32, tag="hsum")
            gGb = pp.tile([4, 256], F32, tag="gGb")
            nc.vector.tensor_reduce(
                hsum[:], hsT[:].rearrange("p (t kb) -> p kb t", kb=16),
                mybir.AxisListType.X, mybir.AluOpType.add)
            with tc.tile_pool(name="p2", bufs=1, space="PSUM") as p2p:
                pg = p2p.tile([4, 256], F32, tag="pg")
                nc.tensor.matmul(pg[:], sm32[0:1, 0:4], bgt[0:1, 0:256],
                                 start=True, stop=False)
                for k in range(4):
                    nc.tensor.matmul(pg[:], hsum[:, 4 * k:4 * k + 4],
                                     wg1t[k][:], start=False, stop=(k == 3))
                nc.scalar.copy(gGb[:], pg[:])

            # ---- Phase 3: Z.T = tanh(Wg2.T.T @ hs.T + gGb bcast) ----
            zt = pp.tile([128, 2 * sb], BF16, tag="zt")
            hs4 = hsT[:].rearrange("p (t k b) -> p k t b", k=4, b=4)
            with tc.tile_pool(name="p3", bufs=2, space="PSUM") as p3p:
                for m in range(2):
                    for n in range(ntb):
                        pz3 = p3p.tile([128, nw], F32, tag="pz3")
                        nc.tensor.matmul(pz3[:], gGb[0:4, 128 * m:128 * (m + 1)],
                                         indrep[0:4, 0:nw], start=True, stop=False)
                        for k in range(4):
                            rhs = hs4[:, k, tpn * n:tpn * (n + 1), :]
                            nc.tensor.matmul(pz3[:],
                                             wg2t[k][:, 128 * m:128 * (m + 1)],
                                             rhs, start=False, stop=(k == 3))
                        nc.scalar.activation(
                            zt[:, m * sb + nw * n: m * sb + nw * (n + 1)],
                            pz3[:], TANH)

            # ---- Phase 4: tag.T = Wtag.T.T @ Z.T + b_tag ----
            tagT = pp.tile([128, sb], F32, tag="tagT")
            with tc.tile_pool(name="p4", bufs=2, space="PSUM") as p4p:
                for n in range(ntb):
                    pz4 = p4p.tile([20, nw], F32, tag="pz4")
                    nc.tensor.matmul(pz4[:], btag[0:1, 0:20],
                                     ones512[0:1, 0:nw], start=True, stop=False)
                    for k in range(2):
                        nc.tensor.matmul(
                            pz4[:], wtagt[k][:],
                            zt[:, k * sb + nw * n: k * sb + nw * (n + 1)],
                            start=False, stop=(k == 1))
                    nc.scalar.copy(tagT[0:20, nw * n:nw * (n + 1)], pz4[:])

            # ---- Phase 5: log-softmax over batch ----
            etag = pp.tile([128, sb], F32, tag="etag")
            sums = pp.tile([128, steps], F32, tag="sums")
            nc.scalar.activation(etag[0:20, :], tagT[0:20, :],
                                 mybir.ActivationFunctionType.Exp)
            nc.vector.tensor_reduce(
                sums[0:20, :],
                etag[0:20, :].rearrange("p (t b) -> p t b", b=BL),
                mybir.AxisListType.X, mybir.AluOpType.add)
            cc_in = dp.tile([20, steps], F32, tag="cc_in")
            cc_out = dp.tile([20, steps], F32, tag="cc_out")
            nc.gpsimd.dma_start(cc_in[:], sums[0:20, :])
            nc.gpsimd.collective_compute(
                "AllReduce", mybir.AluOpType.add,
                replica_groups=[list(range(NCORES))],
                ins=[cc_in[:].opt()], outs=[cc_out[:].opt()])
            nc.gpsimd.dma_start(sums[0:20, :], cc_out[:])
            nc.scalar.activation(sums[0:20, :], sums[0:20, :],
                                 mybir.ActivationFunctionType.Ln)
            tag3 = tagT[0:20, :].rearrange("p (t b) -> p t b", b=BL)
            for b in range(BL):
                nc.vector.tensor_sub(tag3[:, :, b], tag3[:, :, b],
                                     sums[0:20, :])

            # ---- Phase 6: transpose (20, sb) -> (sb, 20) and write out ----
            obuf = pp.tile([tw, ntr * 20], F32, tag="obuf")
            with tc.tile_pool(name="p6", bufs=2, space="PSUM") as p6p:
                for j in range(ntr):
                    pz6 = p6p.tile([tw, 20], F32, tag="pz6")
                    nc.tensor.transpose(pz6[:],
                                        tagT[0:20, tw * j:tw * (j + 1)],
                                        i20[:])
                    nc.scalar.copy(obuf[:, 20 * j:20 * (j + 1)], pz6[:])
            nc.sync.dma_start(
                out_d[:].rearrange("(j p) k -> p j k", p=tw),
                obuf[0:tw, :].rearrange("p (j k) -> p j k", j=ntr))
    nc.finalize()
    return nc


def _prep_inputs(inputs, steps=S):
    """Host-side prep: gather + transpose + pack per-core shards."""
    x = np.asarray(inputs["x"]).astype(np.int64)[:steps]          # (steps, B)
    embed = np.asarray(inputs["embed"], np.float32)
    W_ih = np.asarray(inputs["W_ih"], np.float32)
    W_hh = np.asarray(inputs["W_hh"], np.float32)
    b_ih = np.asarray(inputs["b_ih"], np.float32)
    b_hh = np.asarray(inputs["b_hh"], np.float32)
    W_g = np.asarray(inputs["W_g"], np.float32)
    b_g = np.asarray(inputs["b_g"], np.float32)
    W_tag = np.asarray(inputs["W_tag"], np.float32)
    b_tag = np.asarray(inputs["b_tag"], np.float32)

    sb = steps * BL
    emb = embed[x]                                                # (steps, B, E)
    wihT = W_ih.T.astype(np.float32)                              # (E, 4H)
    bias = (b_ih + b_hh).astype(np.float32)                       # (4H,)

    i4s = np.zeros((128, 4), np.float32)
    i4s[:4, :4] = np.eye(4)
    sm32 = np.zeros((128, 256), np.float32)
    sm32[0, :4] = 1.0
    sm32[32, :256] = b_g
    indrep = np.tile(np.eye(4, dtype=np.float32), (1, 128))
    shared = {
        "ones": np.ones((1, sb), BF16_NP),
        "wih2": bias.reshape(1, 2048).astype(BF16_NP),
        "ones512": np.ones((1, 512), BF16_NP),
        "btag": b_tag.reshape(1, 20).astype(BF16_NP),
        "bgt": b_g.reshape(1, 256).astype(np.float32),
        "wih0": wihT[0:128].astype(BF16_NP),
        "wih1": wihT[128:256].astype(BF16_NP),
        "whh": W_hh.T.astype(np.float32).copy(),
        "i4s": i4s,
        "sm32": sm32,
        "wg1t": W_g[:, :H].T.astype(np.float32).copy(),
        "wg2t": W_g[:, H:].T.astype(BF16_NP),
        "indrep": indrep,
        "wtagt": W_tag.T.astype(BF16_NP),
        "i20": np.eye(20, dtype=np.float32),
    }
    in_maps = []
    for c in range(NCORES):
        sl = emb[:, BL * c:BL * (c + 1), :]                       # (steps, BL, E)
        embT = np.ascontiguousarray(sl.transpose(2, 0, 1).reshape(E, sb))
        m = dict(shared)
        e0p = np.zeros((128, sb + 32), np.float32)
        e0p[:, :sb] = embT[0:128]
        m["e0"] = e0p.astype(BF16_NP)
        m["e1"] = embT[128:256].astype(BF16_NP)
        in_maps.append(m)
    return in_maps


def run(inputs, steps=S, trace=False):
    from concourse.bass_utils import run_bass_kernel_spmd
    key = steps
    if key not in _graph_cache:
        _graph_cache[key] = build_graph(steps)
    nc = _graph_cache[key]
    in_maps = _prep_inputs(inputs, steps)
    res = run_bass_kernel_spmd(nc, in_maps, core_ids=list(range(NCORES)),
                               trace=trace)
    outs = [r["out"].reshape(steps, BL, T) for r in res.results]
    full = np.concatenate(outs, axis=1).astype(np.float32)        # (steps, B, T)
    return full, res


def kernel(**inputs):
    out, _ = run(inputs, steps=S, trace=False)
    return out


# revision 9
# speedup vs baseline: 4.4976x; 4.4976x over previous
"""Trainium2 Bass kernel for nn_Att_LSTM_67989332296335.

Math note: the reference's attention softmax is over a singleton axis, so
A == 1 identically and G[t] = sum_j hs[j] for every t — the whole (S,S,B)
distance tensor is dead code.  The live computation is: embedding gather ->
1024-step LSTM -> hsum -> Z = tanh(G @ Wg1.T + hs @ Wg2.T + b_g) ->
tag = Z @ W_tag.T + b_tag -> log_softmax over the batch axis.

Distribution: data-parallel over batch B=32 across 8 cores (B_local=4),
per the sharding hint.  The LSTM recurrence runs per-core with a
column-tiled TensorE schedule: col group g owns hidden-unit chunk
U_g = [128g, 128g+128) x all 4 gates; weights stream as the moving operand
(7 K-chunks of N=512 per group per step: 2 emb + 1 bias-row + 4 W_hh) while
the tiny h.T / emb.T slices are stationary.  h -> h.T each step via 16
small col-tiled identity matmuls.  The batch log-softmax needs one 8-core
AllReduce of the (20, S) exp-sums.
"""
import sys
sys.path.insert(0, "/opt/trn_rl_repo")

import numpy as np
import ml_dtypes

import concourse.bass as bass
import concourse.tile as tile
from concourse import bacc, mybir

F32 = mybir.dt.float32
BF16 = mybir.dt.bfloat16
BF16_NP = ml_dtypes.bfloat16

S, B, E, H, AH, V, T = 1024, 32, 256, 512, 256, 32000, 20
NCORES = 8
BL = B // NCORES  # 4

_graph_cache = {}
GPERM = [0, 1, 3, 2]  # column gate-block order becomes [i, f, o, g]


def _perm_gates_cols(w):
    """Permute the 2048-wide gate axis (last) into [i,f,o,g] block order."""
    shp = w.shape
    v = w.reshape(shp[:-1] + (4, 512))
    return np.ascontiguousarray(v[..., GPERM, :].reshape(shp))


def build_graph(steps=S, debug=False):
    nc = bacc.Bacc(None, target_bir_lowering=False, debug=debug)
    sb = steps * BL
    tpn = min(128, steps)                 # timesteps per P3/P4 N-tile
    nw = tpn * BL                         # N width (cols) per tile
    ntb = steps // tpn                    # number of N-tiles
    tw = min(128, sb)                     # output-transpose tile width
    ntr = sb // tw

    def P(name, shape, dt):
        return nc.dram_tensor(name, list(shape), dt, kind="ExternalInput")

    e0_d = P("e0", (128, sb + 32), BF16)
    e1_d = P("e1", (128, sb), BF16)
    ones_d = P("ones", (1, sb), BF16)
    wih2_d = P("wih2", (1, 2048), BF16)
    ones512_d = P("ones512", (1, 512), BF16)
    btag_d = P("btag", (1, 20), BF16)
    bgt_d = P("bgt", (1, 256), F32)
    wih0_d = P("wih0", (128, 2048), BF16)
    wih1_d = P("wih1", (128, 2048), BF16)
    whh_d = P("whh", (512, 2048), BF16)
    i4s_d = P("i4s", (128, 4), F32)
    sm32_d = P("sm32", (128, 256), F32)
    wg1t_d = P("wg1t", (512, 256), F32)
    wg2t_d = P("wg2t", (512, 256), BF16)
    indrep_d = P("indrep", (4, 512), F32)
    wtagt_d = P("wtagt", (256, 20), BF16)
    i20_d = P("i20", (20, 20), F32)
    out_d = nc.dram_tensor("out", [sb, T], F32, kind="ExternalOutput")

    SIG = mybir.ActivationFunctionType.Sigmoid
    TANH = mybir.ActivationFunctionType.Tanh

    def strips(w, g):
        """Moving-operand AP: cols {512*gate + 128*g + [0:128)} of a (p, 2048) tile."""
        return w[:].rearrange("p (G C u) -> p G C u", G=4, C=4)[:, :, g, :]

    with tile.TileContext(nc) as tc:
        with tc.tile_pool(name="persist", bufs=1) as pp, \
             tc.tile_pool(name="dram", bufs=1, space="DRAM") as dp:
            e0 = pp.tile([128, sb + 32], BF16, tag="e0")
            e1 = pp.tile([128, sb], BF16, tag="e1")
            ones = pp.tile([1, sb], BF16, tag="ones")
            wih2 = pp.tile([1, 2048], BF16, tag="wih2")
            ones512 = pp.tile([1, 512], BF16, tag="ones512")
            btag = pp.tile([1, 20], BF16, tag="btag")
            bgt = pp.tile([1, 256], F32, tag="bgt")
            wih0 = pp.tile([128, 2048], BF16, tag="wih0")
            wih1 = pp.tile([128, 2048], BF16, tag="wih1")
            whh = [pp.tile([128, 2048], BF16, name=f"whh{k}", tag=f"whh{k}") for k in range(4)]
            i4s = pp.tile([128, 4], F32, tag="i4s")
            sm32 = pp.tile([128, 256], F32, tag="sm32")
            wg1t = [pp.tile([128, 256], F32, name=f"wg1t{k}", tag=f"wg1t{k}") for k in range(4)]
            wg2t = [pp.tile([128, 256], BF16, name=f"wg2t{k}", tag=f"wg2t{k}") for k in range(4)]
            indrep = pp.tile([4, 512], F32, tag="indrep")
            wtagt = [pp.tile([128, 20], BF16, name=f"wtagt{k}", tag=f"wtagt{k}") for k in range(2)]
            i20 = pp.tile([20, 20], F32, tag="i20")

            hT = pp.tile([128, 16], BF16, tag="hT")
            hsT = pp.tile([128, steps * 16], BF16, tag="hsT")
            cst = pp.tile([128, 128], F32, tag="cst")
            sif = pp.tile([128, 384], F32, tag="sif")
            tg = pp.tile([128, 128], F32, tag="tg")
            so = pp.tile([128, 128], F32, tag="so")
            tcl = pp.tile([128, 128], F32, tag="tcl")
            m1 = pp.tile([128, 128], F32, tag="m1")
            m2 = pp.tile([128, 128], F32, tag="m2")
            hn2 = pp.tile([4, 512], F32, tag="hn2")

            for t_, src in [(e0, e0_d), (e1, e1_d), (ones, ones_d),
                            (wih2, wih2_d), (ones512, ones512_d),
                            (btag, btag_d), (bgt, bgt_d),
                            (wih0, wih0_d), (wih1, wih1_d), (i4s, i4s_d),
                            (sm32, sm32_d), (indrep, indrep_d), (i20, i20_d)]:
                nc.sync.dma_start(t_[:], src[:])
            for k in range(4):
                nc.sync.dma_start(whh[k][:], whh_d[128 * k:128 * (k + 1), :])
                nc.sync.dma_start(wg1t[k][:], wg1t_d[128 * k:128 * (k + 1), :])
                nc.sync.dma_start(wg2t[k][:], wg2t_d[128 * k:128 * (k + 1), :])
            for k in range(2):
                nc.sync.dma_start(wtagt[k][:], wtagt_d[128 * k:128 * (k + 1), :])
            nc.vector.memset(hT[:], 0.0)
            nc.vector.memset(cst[:], 0.0)
            nc.vector.memset(hn2[:], 0.0)

            # ---- Phase 1: LSTM recurrence (fully unrolled) ----
            with tc.tile_pool(name="pz", bufs=2, space="PSUM") as pzp, \
                 tc.tile_pool(name="pt", bufs=2, space="PSUM") as ptp:
                for t in range(steps):
                    pz = pzp.tile([128, 512], F32, tag="pz")
                    # chunk-major across col groups: 4-way tile concurrency
                    for g in range(4):
                        nc.tensor.matmul(pz[32 * g:32 * g + 32, 0:512],
                                         e0[:, 4 * t:4 * t + 32],
                                         strips(wih0, g),
                                         start=True, stop=False,
                                         tile_position=(0, 32 * g),
                                         skip_group_check=True)
                    for g in range(4):
                        nc.tensor.matmul(pz[32 * g:32 * g + 4, 0:512],
                                         e1[:, 4 * t:4 * t + 4],
                                         strips(wih1, g),
                                         start=False, stop=False,
                                         tile_position=(0, 32 * g),
                                         skip_group_check=True)
                    for g in range(4):
                        nc.tensor.matmul(pz[32 * g:32 * g + 4, 0:512],
                                         ones[0:1, 4 * t:4 * t + 4],
                                         strips(wih2, g),
                                         start=False, stop=False,
                                         tile_position=(0, 32 * g),
                                         skip_group_check=True)
                    for k in range(4):
                        for g in range(4):
                            nc.tensor.matmul(pz[32 * g:32 * g + 4, 0:512],
                                             hT[:, 4 * k:4 * k + 4],
                                             strips(whh[k], g),
                                             start=False, stop=(k == 3),
                                             tile_position=(0, 32 * g),
                                             skip_group_check=True)
                    # gates [i|f|o|g]: rows {32g+b} meaningful, full-width ops
                    nc.scalar.activation(sif[:], pz[:, 0:384], SIG)
                    nc.scalar.activation(tg[:], pz[:, 384:512], TANH)
                    nc.vector.tensor_mul(m1[:], sif[:, 128:256], cst[:])
                    nc.vector.tensor_mul(m2[:], sif[:, 0:128], tg[:])
                    nc.vector.tensor_add(cst[:], m1[:], m2[:])
                    nc.scalar.activation(tcl[:], cst[:], TANH)
                    # h to batch-major (4, 512) at partition base 0
                    for g in range(4):
                        nc.vector.tensor_mul(hn2[0:4, 128 * g:128 * (g + 1)],
                                             tcl[32 * g:32 * g + 4, 0:128],
                                             sif[32 * g:32 * g + 4, 256:384])
                    # h -> h.T via 16 tiny col-tiled identity matmuls
                    pt = ptp.tile([128, 16], F32, tag="pt")
                    for g in range(4):
                        for j in range(4):
                            nc.tensor.matmul(
                                pt[32 * j:32 * j + 32, 4 * g:4 * g + 4],
                                hn2[0:4, 128 * g + 32 * j:128 * g + 32 * j + 32],
                                i4s[0:4, 0:4],
                                start=True, stop=True,
                                tile_position=(0, 32 * j))
                    nc.scalar.copy(hT[:], pt[:])
                    nc.scalar.copy(hsT[:, 16 * t:16 * (t + 1)], pt[:])

            # ---- Phase 2: hsum and gGb ----
            hsum = pp.tile([128, 16], F32, tag="hsum")
            gGb = pp.tile([4, 256], F32, tag="gGb")
            nc.vector.tensor_reduce(
                hsum[:], hsT[:].rearrange("p (t kb) -> p kb t", kb=16),
                mybir.AxisListType.X, mybir.AluOpType.add)
            with tc.tile_pool(name="p2", bufs=1, space="PSUM") as p2p:
                pg = p2p.tile([4, 256], F32, tag="pg")
                nc.tensor.matmul(pg[:], sm32[0:1, 0:4], bgt[0:1, 0:256],
                                 start=True, stop=False)
                for k in range(4):
                    nc.tensor.matmul(pg[:], hsum[:, 4 * k:4 * k + 4],
                                     wg1t[k][:], start=False, stop=(k == 3))
                nc.scalar.copy(gGb[:], pg[:])

            # ---- Phase 3: Z.T = tanh(Wg2.T.T @ hs.T + gGb bcast) ----
            zt = pp.tile([128, 2 * sb], BF16, tag="zt")
            hs4 = hsT[:].rearrange("p (t k b) -> p k t b", k=4, b=4)
            with tc.tile_pool(name="p3", bufs=2, space="PSUM") as p3p:
                for m in range(2):
                    for n in range(ntb):
                        pz3 = p3p.tile([128, nw], F32, tag="pz3")
                        nc.tensor.matmul(pz3[:], gGb[0:4, 128 * m:128 * (m + 1)],
                                         indrep[0:4, 0:nw], start=True, stop=False)
                        for k in range(4):
                            rhs = hs4[:, k, tpn * n:tpn * (n + 1), :]
                            nc.tensor.matmul(pz3[:],
                                             wg2t[k][:, 128 * m:128 * (m + 1)],
                                             rhs, start=False, stop=(k == 3))
                        nc.scalar.activation(
                            zt[:, m * sb + nw * n: m * sb + nw * (n + 1)],
                            pz3[:], TANH)

            # ---- Phase 4: tag.T = Wtag.T.T @ Z.T + b_tag ----
            tagT = pp.tile([128, sb], F32, tag="tagT")
            with tc.tile_pool(name="p4", bufs=2, space="PSUM") as p4p:
                for n in range(ntb):
                    pz4 = p4p.tile([20, nw], F32, tag="pz4")
                    nc.tensor.matmul(pz4[:], btag[0:1, 0:20],
                                     ones512[0:1, 0:nw], start=True, stop=False)
                    for k in range(2):
                        nc.tensor.matmul(
                            pz4[:], wtagt[k][:],
                            zt[:, k * sb + nw * n: k * sb + nw * (n + 1)],
                            start=False, stop=(k == 1))
                    nc.scalar.copy(tagT[0:20, nw * n:nw * (n + 1)], pz4[:])

            # ---- Phase 5: log-softmax over batch ----
            etag = pp.tile([128, sb], F32, tag="etag")
            sums = pp.tile([128, steps], F32, tag="sums")
            nc.scalar.activation(etag[0:20, :], tagT[0:20, :],
                                 mybir.ActivationFunctionType.Exp)
            nc.vector.tensor_reduce(
                sums[0:20, :],
                etag[0:20, :].rearrange("p (t b) -> p t b", b=BL),
                mybir.AxisListType.X, mybir.AluOpType.add)
            cc_in = dp.tile([20, steps], F32, tag="cc_in")
            cc_out = dp.tile([20, steps], F32, tag="cc_out")
            nc.gpsimd.dma_start(cc_in[:], sums[0:20, :])
            nc.gpsimd.collective_compute(
                "AllReduce", mybir.AluOpType.add,
                replica_groups=[list(range(NCORES))],
                ins=[cc_in[:].opt()], outs=[cc_out[:].opt()])
            nc.gpsimd.dma_start(sums[0:20, :], cc_out[:])
            nc.scalar.activation(sums[0:20, :], sums[0:20, :],
                                 mybir.ActivationFunctionType.Ln)
            tag3 = tagT[0:20, :].rearrange("p (t b) -> p t b", b=BL)
            for b in range(BL):
                nc.vector.tensor_sub(tag3[:, :, b], tag3[:, :, b],
                                     sums[0:20, :])

            # ---- Phase 6: transpose (20, sb) -> (sb, 20) and write out ----
            obuf = pp.tile([tw, ntr * 20], F32, tag="obuf")
            with tc.tile_pool(name="p6", bufs=2, space="PSUM") as p6p:
                for j in range(ntr):
                    pz6 = p6p.tile([tw, 20], F32, tag="pz6")
                    nc.tensor.transpose(pz6[:],
                                        tagT[0:20, tw * j:tw * (j + 1)],
                                        i20[:])
                    nc.scalar.copy(obuf[:, 20 * j:20 * (j + 1)], pz6[:])
            nc.sync.dma_start(
                out_d[:].rearrange("(j p) k -> p j k", p=tw),
                obuf[0:tw, :].rearrange("p (j k) -> p j k", j=ntr))
    nc.finalize()
    return nc


def _prep_inputs(inputs, steps=S):
    """Host-side prep: gather + transpose + pack per-core shards."""
    x = np.asarray(inputs["x"]).astype(np.int64)[:steps]          # (steps, B)
    embed = np.asarray(inputs["embed"], np.float32)
    W_ih = np.asarray(inputs["W_ih"], np.float32)
    W_hh = np.asarray(inputs["W_hh"], np.float32)
    b_ih = np.asarray(inputs["b_ih"], np.float32)
    b_hh = np.asarray(inputs["b_hh"], np.float32)
    W_g = np.asarray(inputs["W_g"], np.float32)
    b_g = np.asarray(inputs["b_g"], np.float32)
    W_tag = np.asarray(inputs["W_tag"], np.float32)
    b_tag = np.asarray(inputs["b_tag"], np.float32)

    sb = steps * BL
    emb = embed[x]                                                # (steps, B, E)
    wihT = _perm_gates_cols(W_ih.T.astype(np.float32))            # (E, 4H)
    bias = _perm_gates_cols((b_ih + b_hh).astype(np.float32))     # (4H,)

    i4s = np.zeros((128, 4), np.float32)
    i4s[:4, :4] = np.eye(4)
    sm32 = np.zeros((128, 256), np.float32)
    sm32[0, :4] = 1.0
    sm32[32, :256] = b_g
    indrep = np.tile(np.eye(4, dtype=np.float32), (1, 128))
    shared = {
        "ones": np.ones((1, sb), BF16_NP),
        "wih2": bias.reshape(1, 2048).astype(BF16_NP),
        "ones512": np.ones((1, 512), BF16_NP),
        "btag": b_tag.reshape(1, 20).astype(BF16_NP),
        "bgt": b_g.reshape(1, 256).astype(np.float32),
        "wih0": wihT[0:128].astype(BF16_NP),
        "wih1": wihT[128:256].astype(BF16_NP),
        "whh": _perm_gates_cols(W_hh.T.astype(np.float32)).astype(BF16_NP),
        "i4s": i4s,
        "sm32": sm32,
        "wg1t": W_g[:, :H].T.astype(np.float32).copy(),
        "wg2t": W_g[:, H:].T.astype(BF16_NP),
        "indrep": indrep,
        "wtagt": W_tag.T.astype(BF16_NP),
        "i20": np.eye(20, dtype=np.float32),
    }
    in_maps = []
    for c in range(NCORES):
        sl = emb[:, BL * c:BL * (c + 1), :]                       # (steps, BL, E)
        embT = np.ascontiguousarray(sl.transpose(2, 0, 1).reshape(E, sb))
        m = dict(shared)
        e0p = np.zeros((128, sb + 32), np.float32)
        e0p[:, :sb] = embT[0:128]
        m["e0"] = e0p.astype(BF16_NP)
        m["e1"] = embT[128:256].astype(BF16_NP)
        in_maps.append(m)
    return in_maps


def run(inputs, steps=S, trace=False):
    from concourse.bass_utils import run_bass_kernel_spmd
    key = steps
    if key not in _graph_cache:
        _graph_cache[key] = build_graph(steps)
    nc = _graph_cache[key]
    in_maps = _prep_inputs(inputs, steps)
    res = run_bass_kernel_spmd(nc, in_maps, core_ids=list(range(NCORES)),
                               trace=trace)
    outs = [r["out"].reshape(steps, BL, T) for r in res.results]
    full = np.concatenate(outs, axis=1).astype(np.float32)        # (steps, B, T)
    return full, res


def kernel(**inputs):
    out, _ = run(inputs, steps=S, trace=False)
    return out


# revision 11
# speedup vs baseline: 4.5078x; 1.0023x over previous
"""Trainium2 Bass kernel for nn_Att_LSTM_67989332296335.

Math note: the reference's attention softmax is over a singleton axis, so
A == 1 identically and G[t] = sum_j hs[j] for every t — the whole (S,S,B)
distance tensor is dead code.  The live computation is: embedding gather ->
1024-step LSTM -> hsum -> Z = tanh(G @ Wg1.T + hs @ Wg2.T + b_g) ->
tag = Z @ W_tag.T + b_tag -> log_softmax over the batch axis.

Distribution: data-parallel over batch B=32 across 8 cores (B_local=4),
per the sharding hint.  The LSTM recurrence runs per-core with a
column-tiled TensorE schedule: col group g owns hidden-unit chunk
U_g = [128g, 128g+128) x all 4 gates; weights stream as the moving operand
(7 K-chunks of N=512 per group per step: 2 emb + 1 bias-row + 4 W_hh) while
the tiny h.T / emb.T slices are stationary.  h -> h.T each step via 16
small col-tiled identity matmuls.  The batch log-softmax needs one 8-core
AllReduce of the (20, S) exp-sums.
"""
import sys
sys.path.insert(0, "/opt/trn_rl_repo")

import numpy as np
import ml_dtypes

import concourse.bass as bass
import concourse.tile as tile
from concourse import bacc, mybir

F32 = mybir.dt.float32
BF16 = mybir.dt.bfloat16
BF16_NP = ml_dtypes.bfloat16

S, B, E, H, AH, V, T = 1024, 32, 256, 512, 256, 32000, 20
NCORES = 8
BL = B // NCORES  # 4

_graph_cache = {}
GPERM = [0, 1, 3, 2]  # column gate-block order becomes [i, f, o, g]


def _perm_gates_cols(w):
    """Permute the 2048-wide gate axis (last) into [i,f,o,g] block order."""
    shp = w.shape
    v = w.reshape(shp[:-1] + (4, 512))
    return np.ascontiguousarray(v[..., GPERM, :].reshape(shp))


def build_graph(steps=S, debug=False):
    nc = bacc.Bacc(None, target_bir_lowering=False, debug=debug)
    sb = steps * BL
    tpn = min(128, steps)                 # timesteps per P3/P4 N-tile
    nw = tpn * BL                         # N width (cols) per tile
    ntb = steps // tpn                    # number of N-tiles
    tw = min(128, sb)                     # output-transpose tile width
    ntr = sb // tw

    def P(name, shape, dt):
        return nc.dram_tensor(name, list(shape), dt, kind="ExternalInput")

    e0_d = P("e0", (128, sb + 32), BF16)
    e1_d = P("e1", (128, sb), BF16)
    ones_d = P("ones", (1, sb), BF16)
    wih2_d = P("wih2", (1, 2048), BF16)
    ones512_d = P("ones512", (1, 512), BF16)
    btag_d = P("btag", (1, 20), BF16)
    bgt_d = P("bgt", (1, 256), F32)
    wih0_d = P("wih0", (128, 2048), BF16)
    wih1_d = P("wih1", (128, 2048), BF16)
    whh_d = P("whh", (512, 2048), BF16)
    i4s_d = P("i4s", (128, 4), F32)
    sm32_d = P("sm32", (128, 256), F32)
    wg1t_d = P("wg1t", (512, 256), F32)
    wg2t_d = P("wg2t", (512, 256), BF16)
    indrep_d = P("indrep", (4, 512), F32)
    wtagt_d = P("wtagt", (256, 20), BF16)
    i20_d = P("i20", (20, 20), F32)
    out_d = nc.dram_tensor("out", [sb, T], F32, kind="ExternalOutput")

    SIG = mybir.ActivationFunctionType.Sigmoid
    TANH = mybir.ActivationFunctionType.Tanh

    def strips(w, g):
        """Moving-operand AP: cols {512*gate + 128*g + [0:128)} of a (p, 2048) tile."""
        return w[:].rearrange("p (G C u) -> p G C u", G=4, C=4)[:, :, g, :]

    with tile.TileContext(nc) as tc:
        with tc.tile_pool(name="persist", bufs=1) as pp, \
             tc.tile_pool(name="dram", bufs=1, space="DRAM") as dp:
            e0 = pp.tile([128, sb + 32], BF16, tag="e0")
            e1 = pp.tile([128, sb], BF16, tag="e1")
            ones = pp.tile([1, sb], BF16, tag="ones")
            wih2 = pp.tile([1, 2048], BF16, tag="wih2")
            ones512 = pp.tile([1, 512], BF16, tag="ones512")
            btag = pp.tile([1, 20], BF16, tag="btag")
            bgt = pp.tile([1, 256], F32, tag="bgt")
            wih0 = pp.tile([128, 2048], BF16, tag="wih0")
            wih1 = pp.tile([128, 2048], BF16, tag="wih1")
            whh = [pp.tile([128, 2048], BF16, name=f"whh{k}", tag=f"whh{k}") for k in range(4)]
            i4s = pp.tile([128, 4], F32, tag="i4s")
            sm32 = pp.tile([128, 256], F32, tag="sm32")
            wg1t = [pp.tile([128, 256], F32, name=f"wg1t{k}", tag=f"wg1t{k}") for k in range(4)]
            wg2t = [pp.tile([128, 256], BF16, name=f"wg2t{k}", tag=f"wg2t{k}") for k in range(4)]
            indrep = pp.tile([4, 512], F32, tag="indrep")
            wtagt = [pp.tile([128, 20], BF16, name=f"wtagt{k}", tag=f"wtagt{k}") for k in range(2)]
            i20 = pp.tile([20, 20], F32, tag="i20")

            hT = pp.tile([128, 16], BF16, tag="hT")
            hsT = pp.tile([128, steps * 16], BF16, tag="hsT")
            cst = pp.tile([128, 128], F32, tag="cst")
            sif = pp.tile([128, 384], F32, tag="sif")
            tg = pp.tile([128, 128], F32, tag="tg")
            so = pp.tile([128, 128], F32, tag="so")
            tcl = pp.tile([128, 128], F32, tag="tcl")
            m1 = pp.tile([128, 128], F32, tag="m1")
            m2 = pp.tile([128, 128], F32, tag="m2")
            hn2 = pp.tile([4, 512], F32, tag="hn2")

            for t_, src in [(e0, e0_d), (e1, e1_d), (ones, ones_d),
                            (wih2, wih2_d), (ones512, ones512_d),
                            (btag, btag_d), (bgt, bgt_d),
                            (wih0, wih0_d), (wih1, wih1_d), (i4s, i4s_d),
                            (sm32, sm32_d), (indrep, indrep_d), (i20, i20_d)]:
                nc.sync.dma_start(t_[:], src[:])
            for k in range(4):
                nc.sync.dma_start(whh[k][:], whh_d[128 * k:128 * (k + 1), :])
                nc.sync.dma_start(wg1t[k][:], wg1t_d[128 * k:128 * (k + 1), :])
                nc.sync.dma_start(wg2t[k][:], wg2t_d[128 * k:128 * (k + 1), :])
            for k in range(2):
                nc.sync.dma_start(wtagt[k][:], wtagt_d[128 * k:128 * (k + 1), :])
            nc.vector.memset(hT[:], 0.0)
            nc.vector.memset(cst[:], 0.0)
            nc.vector.memset(hn2[:], 0.0)

            # ---- Phase 1: LSTM recurrence (fully unrolled) ----
            with tc.tile_pool(name="pz", bufs=2, space="PSUM") as pzp, \
                 tc.tile_pool(name="pt", bufs=2, space="PSUM") as ptp:
                def emb_quads(t, pz):
                    for g in range(4):
                        nc.tensor.matmul(pz[32 * g:32 * g + 32, 0:512],
                                         e0[:, 4 * t:4 * t + 32],
                                         strips(wih0, g),
                                         start=True, stop=False,
                                         tile_position=(0, 32 * g),
                                         skip_group_check=True)
                    for g in range(4):
                        nc.tensor.matmul(pz[32 * g:32 * g + 4, 0:512],
                                         e1[:, 4 * t:4 * t + 4],
                                         strips(wih1, g),
                                         start=False, stop=False,
                                         tile_position=(0, 32 * g),
                                         skip_group_check=True)
                    for g in range(4):
                        nc.tensor.matmul(pz[32 * g:32 * g + 4, 0:512],
                                         ones[0:1, 4 * t:4 * t + 4],
                                         strips(wih2, g),
                                         start=False, stop=False,
                                         tile_position=(0, 32 * g),
                                         skip_group_check=True)

                pz_cur = pzp.tile([128, 512], F32, tag="pz", name="pz0")
                emb_quads(0, pz_cur)
                for t in range(steps):
                    pz = pz_cur
                    for k in range(4):
                        for g in range(4):
                            nc.tensor.matmul(pz[32 * g:32 * g + 4, 0:512],
                                             hT[:, 4 * k:4 * k + 4],
                                             strips(whh[k], g),
                                             start=False, stop=(k == 3),
                                             tile_position=(0, 32 * g),
                                             skip_group_check=True)
                    # software pipelining: next step's emb/bias matmuls are
                    # independent of h_t — issue them before the elementwise
                    # so the PE works through them while ACT/DVE run.
                    if t + 1 < steps:
                        pz_cur = pzp.tile([128, 512], F32, tag="pz",
                                          name=f"pz{t + 1}")
                        emb_quads(t + 1, pz_cur)
                    # gates [i|f|o|g]: rows {32g+b} meaningful, full-width ops
                    nc.scalar.activation(sif[:], pz[:, 0:384], SIG)
                    nc.scalar.activation(tg[:], pz[:, 384:512], TANH)
                    nc.vector.tensor_mul(m1[:], sif[:, 128:256], cst[:])
                    nc.gpsimd.tensor_mul(m2[:], sif[:, 0:128], tg[:])
                    nc.vector.tensor_add(cst[:], m1[:], m2[:])
                    nc.scalar.activation(tcl[:], cst[:], TANH)
                    # h to batch-major (4, 512) at partition base 0
                    for g in range(4):
                        eng = nc.vector if g % 2 == 0 else nc.gpsimd
                        eng.tensor_mul(hn2[0:4, 128 * g:128 * (g + 1)],
                                       tcl[32 * g:32 * g + 4, 0:128],
                                       sif[32 * g:32 * g + 4, 256:384])
                    # h -> h.T via 16 tiny col-tiled identity matmuls
                    pt = ptp.tile([128, 16], F32, tag="pt")
                    for g in range(4):
                        for j in range(4):
                            nc.tensor.matmul(
                                pt[32 * j:32 * j + 32, 4 * g:4 * g + 4],
                                hn2[0:4, 128 * g + 32 * j:128 * g + 32 * j + 32],
                                i4s[0:4, 0:4],
                                start=True, stop=True,
                                tile_position=(0, 32 * j))
                    for k in range(4):
                        nc.scalar.copy(hT[:, 4 * k:4 * k + 4],
                                       pt[:, 4 * k:4 * k + 4])
                    nc.scalar.copy(hsT[:, 16 * t:16 * (t + 1)], pt[:])

            # ---- Phase 2: hsum and gGb ----
            hsum = pp.tile([128, 16], F32, tag="hsum")
            gGb = pp.tile([4, 256], F32, tag="gGb")
            nc.vector.tensor_reduce(
                hsum[:], hsT[:].rearrange("p (t kb) -> p kb t", kb=16),
                mybir.AxisListType.X, mybir.AluOpType.add)
            with tc.tile_pool(name="p2", bufs=1, space="PSUM") as p2p:
                pg = p2p.tile([4, 256], F32, tag="pg")
                nc.tensor.matmul(pg[:], sm32[0:1, 0:4], bgt[0:1, 0:256],
                                 start=True, stop=False)
                for k in range(4):
                    nc.tensor.matmul(pg[:], hsum[:, 4 * k:4 * k + 4],
                                     wg1t[k][:], start=False, stop=(k == 3))
                nc.scalar.copy(gGb[:], pg[:])

            # ---- Phase 3: Z.T = tanh(Wg2.T.T @ hs.T + gGb bcast) ----
            zt = pp.tile([128, 2 * sb], BF16, tag="zt")
            hs4 = hsT[:].rearrange("p (t k b) -> p k t b", k=4, b=4)
            with tc.tile_pool(name="p3", bufs=2, space="PSUM") as p3p:
                for m in range(2):
                    for n in range(ntb):
                        pz3 = p3p.tile([128, nw], F32, tag="pz3")
                        nc.tensor.matmul(pz3[:], gGb[0:4, 128 * m:128 * (m + 1)],
                                         indrep[0:4, 0:nw], start=True, stop=False)
                        for k in range(4):
                            rhs = hs4[:, k, tpn * n:tpn * (n + 1), :]
                            nc.tensor.matmul(pz3[:],
                                             wg2t[k][:, 128 * m:128 * (m + 1)],
                                             rhs, start=False, stop=(k == 3))
                        nc.scalar.activation(
                            zt[:, m * sb + nw * n: m * sb + nw * (n + 1)],
                            pz3[:], TANH)

            # ---- Phase 4: tag.T = Wtag.T.T @ Z.T + b_tag ----
            tagT = pp.tile([128, sb], F32, tag="tagT")
            with tc.tile_pool(name="p4", bufs=2, space="PSUM") as p4p:
                for n in range(ntb):
                    pz4 = p4p.tile([20, nw], F32, tag="pz4")
                    nc.tensor.matmul(pz4[:], btag[0:1, 0:20],
                                     ones512[0:1, 0:nw], start=True, stop=False)
                    for k in range(2):
                        nc.tensor.matmul(
                            pz4[:], wtagt[k][:],
                            zt[:, k * sb + nw * n: k * sb + nw * (n + 1)],
                            start=False, stop=(k == 1))
                    nc.scalar.copy(tagT[0:20, nw * n:nw * (n + 1)], pz4[:])

            # ---- Phase 5: log-softmax over batch ----
            etag = pp.tile([128, sb], F32, tag="etag")
            sums = pp.tile([128, steps], F32, tag="sums")
            nc.scalar.activation(etag[0:20, :], tagT[0:20, :],
                                 mybir.ActivationFunctionType.Exp)
            nc.vector.tensor_reduce(
                sums[0:20, :],
                etag[0:20, :].rearrange("p (t b) -> p t b", b=BL),
                mybir.AxisListType.X, mybir.AluOpType.add)
            cc_in = dp.tile([20, steps], F32, tag="cc_in")
            cc_out = dp.tile([20, steps], F32, tag="cc_out")
            nc.gpsimd.dma_start(cc_in[:], sums[0:20, :])
            nc.gpsimd.collective_compute(
                "AllReduce", mybir.AluOpType.add,
                replica_groups=[list(range(NCORES))],
                ins=[cc_in[:].opt()], outs=[cc_out[:].opt()])
            nc.gpsimd.dma_start(sums[0:20, :], cc_out[:])
            nc.scalar.activation(sums[0:20, :], sums[0:20, :],
                                 mybir.ActivationFunctionType.Ln)
            tag3 = tagT[0:20, :].rearrange("p (t b) -> p t b", b=BL)
            for b in range(BL):
                nc.vector.tensor_sub(tag3[:, :, b], tag3[:, :, b],
                                     sums[0:20, :])

            # ---- Phase 6: transpose (20, sb) -> (sb, 20) and write out ----
            obuf = pp.tile([tw, ntr * 20], F32, tag="obuf")
            with tc.tile_pool(name="p6", bufs=2, space="PSUM") as p6p:
                for j in range(ntr):
                    pz6 = p6p.tile([tw, 20], F32, tag="pz6")
                    nc.tensor.transpose(pz6[:],
                                        tagT[0:20, tw * j:tw * (j + 1)],
                                        i20[:])
                    nc.scalar.copy(obuf[:, 20 * j:20 * (j + 1)], pz6[:])
            nc.sync.dma_start(
                out_d[:].rearrange("(j p) k -> p j k", p=tw),
                obuf[0:tw, :].rearrange("p (j k) -> p j k", j=ntr))
    nc.finalize()
    return nc


def _prep_inputs(inputs, steps=S):
    """Host-side prep: gather + transpose + pack per-core shards."""
    x = np.asarray(inputs["x"]).astype(np.int64)[:steps]          # (steps, B)
    embed = np.asarray(inputs["embed"], np.float32)
    W_ih = np.asarray(inputs["W_ih"], np.float32)
    W_hh = np.asarray(inputs["W_hh"], np.float32)
    b_ih = np.asarray(inputs["b_ih"], np.float32)
    b_hh = np.asarray(inputs["b_hh"], np.float32)
    W_g = np.asarray(inputs["W_g"], np.float32)
    b_g = np.asarray(inputs["b_g"], np.float32)
    W_tag = np.asarray(inputs["W_tag"], np.float32)
    b_tag = np.asarray(inputs["b_tag"], np.float32)

    sb = steps * BL
    emb = embed[x]                                                # (steps, B, E)
    wihT = _perm_gates_cols(W_ih.T.astype(np.float32))            # (E, 4H)
    bias = _perm_gates_cols((b_ih + b_hh).astype(np.float32))     # (4H,)

    i4s = np.zeros((128, 4), np.float32)
    i4s[:4, :4] = np.eye(4)
    sm32 = np.zeros((128, 256), np.float32)
    sm32[0, :4] = 1.0
    sm32[32, :256] = b_g
    indrep = np.tile(np.eye(4, dtype=np.float32), (1, 128))
    shared = {
        "ones": np.ones((1, sb), BF16_NP),
        "wih2": bias.reshape(1, 2048).astype(BF16_NP),
        "ones512": np.ones((1, 512), BF16_NP),
        "btag": b_tag.reshape(1, 20).astype(BF16_NP),
        "bgt": b_g.reshape(1, 256).astype(np.float32),
        "wih0": wihT[0:128].astype(BF16_NP),
        "wih1": wihT[128:256].astype(BF16_NP),
        "whh": _perm_gates_cols(W_hh.T.astype(np.float32)).astype(BF16_NP),
        "i4s": i4s,
        "sm32": sm32,
        "wg1t": W_g[:, :H].T.astype(np.float32).copy(),
        "wg2t": W_g[:, H:].T.astype(BF16_NP),
        "indrep": indrep,
        "wtagt": W_tag.T.astype(BF16_NP),
        "i20": np.eye(20, dtype=np.float32),
    }
    in_maps = []
    for c in range(NCORES):
        sl = emb[:, BL * c:BL * (c + 1), :]                       # (steps, BL, E)
        embT = np.ascontiguousarray(sl.transpose(2, 0, 1).reshape(E, sb))
        m = dict(shared)
        e0p = np.zeros((128, sb + 32), np.float32)
        e0p[:, :sb] = embT[0:128]
        m["e0"] = e0p.astype(BF16_NP)
        m["e1"] = embT[128:256].astype(BF16_NP)
        in_maps.append(m)
    return in_maps


def run(inputs, steps=S, trace=False):
    from concourse.bass_utils import run_bass_kernel_spmd
    key = steps
    if key not in _graph_cache:
        _graph_cache[key] = build_graph(steps)
    nc = _graph_cache[key]
    in_maps = _prep_inputs(inputs, steps)
    res = run_bass_kernel_spmd(nc, in_maps, core_ids=list(range(NCORES)),
                               trace=trace)
    outs = [r["out"].reshape(steps, BL, T) for r in res.results]
    full = np.concatenate(outs, axis=1).astype(np.float32)        # (steps, B, T)
    return full, res


def kernel(**inputs):
    out, _ = run(inputs, steps=S, trace=False)
    return out


# revision 12
# speedup vs baseline: 4.5288x; 1.0047x over previous
"""Trainium2 Bass kernel for nn_Att_LSTM_67989332296335.

Math note: the reference's attention softmax is over a singleton axis, so
A == 1 identically and G[t] = sum_j hs[j] for every t — the whole (S,S,B)
distance tensor is dead code.  The live computation is: embedding gather ->
1024-step LSTM -> hsum -> Z = tanh(G @ Wg1.T + hs @ Wg2.T + b_g) ->
tag = Z @ W_tag.T + b_tag -> log_softmax over the batch axis.

Distribution: data-parallel over batch B=32 across 8 cores (B_local=4),
per the sharding hint.  The LSTM recurrence runs per-core with a
column-tiled TensorE schedule: col group g owns hidden-unit chunk
U_g = [128g, 128g+128) x all 4 gates; weights stream as the moving operand
(7 K-chunks of N=512 per group per step: 2 emb + 1 bias-row + 4 W_hh) while
the tiny h.T / emb.T slices are stationary.  h -> h.T each step via 16
small col-tiled identity matmuls.  The batch log-softmax needs one 8-core
AllReduce of the (20, S) exp-sums.
"""
import sys
sys.path.insert(0, "/opt/trn_rl_repo")

import numpy as np
import ml_dtypes

import concourse.bass as bass
import concourse.tile as tile
from concourse import bacc, mybir

F32 = mybir.dt.float32
BF16 = mybir.dt.bfloat16
BF16_NP = ml_dtypes.bfloat16

S, B, E, H, AH, V, T = 1024, 32, 256, 512, 256, 32000, 20
NCORES = 8
BL = B // NCORES  # 4

_graph_cache = {}
GPERM = [0, 1, 3, 2]  # column gate-block order becomes [i, f, o, g]


def _perm_gates_cols(w):
    """Permute the 2048-wide gate axis (last) into [i,f,o,g] block order."""
    shp = w.shape
    v = w.reshape(shp[:-1] + (4, 512))
    return np.ascontiguousarray(v[..., GPERM, :].reshape(shp))


def build_graph(steps=S, debug=False):
    nc = bacc.Bacc(None, target_bir_lowering=False, debug=debug)
    sb = steps * BL
    tpn = min(128, steps)                 # timesteps per P3/P4 N-tile
    nw = tpn * BL                         # N width (cols) per tile
    ntb = steps // tpn                    # number of N-tiles
    tw = min(128, sb)                     # output-transpose tile width
    ntr = sb // tw

    def P(name, shape, dt):
        return nc.dram_tensor(name, list(shape), dt, kind="ExternalInput")

    e0_d = P("e0", (128, sb + 32), BF16)
    e1_d = P("e1", (128, sb), BF16)
    ones_d = P("ones", (1, sb), BF16)
    wih2_d = P("wih2", (1, 2048), BF16)
    ones512_d = P("ones512", (1, 512), BF16)
    btag_d = P("btag", (1, 20), BF16)
    bgt_d = P("bgt", (1, 256), F32)
    wih0_d = P("wih0", (128, 2048), BF16)
    wih1_d = P("wih1", (128, 2048), BF16)
    whh_d = P("whh", (512, 2048), BF16)
    i4s_d = P("i4s", (128, 4), F32)
    sm32_d = P("sm32", (128, 256), F32)
    wg1t_d = P("wg1t", (512, 256), F32)
    wg2t_d = P("wg2t", (512, 256), BF16)
    indrep_d = P("indrep", (4, 512), F32)
    wtagt_d = P("wtagt", (256, 20), BF16)
    i20_d = P("i20", (20, 20), F32)
    out_d = nc.dram_tensor("out", [sb, T], F32, kind="ExternalOutput")

    SIG = mybir.ActivationFunctionType.Sigmoid
    TANH = mybir.ActivationFunctionType.Tanh

    def strips(w, g):
        """Moving-operand AP: cols {512*gate + 128*g + [0:128)} of a (p, 2048) tile."""
        return w[:].rearrange("p (G C u) -> p G C u", G=4, C=4)[:, :, g, :]

    with tile.TileContext(nc) as tc:
        with tc.tile_pool(name="persist", bufs=1) as pp, \
             tc.tile_pool(name="dram", bufs=1, space="DRAM") as dp:
            e0 = pp.tile([128, sb + 32], BF16, tag="e0")
            e1 = pp.tile([128, sb], BF16, tag="e1")
            ones = pp.tile([1, sb], BF16, tag="ones")
            wih2 = pp.tile([1, 2048], BF16, tag="wih2")
            ones512 = pp.tile([1, 512], BF16, tag="ones512")
            btag = pp.tile([1, 20], BF16, tag="btag")
            bgt = pp.tile([1, 256], F32, tag="bgt")
            wih0 = pp.tile([128, 2048], BF16, tag="wih0")
            wih1 = pp.tile([128, 2048], BF16, tag="wih1")
            whh = [pp.tile([128, 2048], BF16, name=f"whh{k}", tag=f"whh{k}") for k in range(4)]
            i4s = pp.tile([128, 4], F32, tag="i4s")
            sm32 = pp.tile([128, 256], F32, tag="sm32")
            wg1t = [pp.tile([128, 256], F32, name=f"wg1t{k}", tag=f"wg1t{k}") for k in range(4)]
            wg2t = [pp.tile([128, 256], BF16, name=f"wg2t{k}", tag=f"wg2t{k}") for k in range(4)]
            indrep = pp.tile([4, 512], F32, tag="indrep")
            wtagt = [pp.tile([128, 20], BF16, name=f"wtagt{k}", tag=f"wtagt{k}") for k in range(2)]
            i20 = pp.tile([20, 20], F32, tag="i20")

            hT = pp.tile([128, 16], BF16, tag="hT")
            hsT = pp.tile([128, steps * 16], BF16, tag="hsT")
            cst = pp.tile([128, 128], F32, tag="cst")
            sif = pp.tile([128, 384], F32, tag="sif")
            tg = pp.tile([128, 128], F32, tag="tg")
            so = pp.tile([128, 128], F32, tag="so")
            tcl = pp.tile([128, 128], F32, tag="tcl")
            m1 = pp.tile([128, 128], F32, tag="m1")
            m2 = pp.tile([128, 128], F32, tag="m2")
            hn2 = pp.tile([4, 512], F32, tag="hn2")

            for t_, src in [(e0, e0_d), (e1, e1_d), (ones, ones_d),
                            (wih2, wih2_d), (ones512, ones512_d),
                            (btag, btag_d), (bgt, bgt_d),
                            (wih0, wih0_d), (wih1, wih1_d), (i4s, i4s_d),
                            (sm32, sm32_d), (indrep, indrep_d), (i20, i20_d)]:
                nc.sync.dma_start(t_[:], src[:])
            for k in range(4):
                nc.sync.dma_start(whh[k][:], whh_d[128 * k:128 * (k + 1), :])
                nc.sync.dma_start(wg1t[k][:], wg1t_d[128 * k:128 * (k + 1), :])
                nc.sync.dma_start(wg2t[k][:], wg2t_d[128 * k:128 * (k + 1), :])
            for k in range(2):
                nc.sync.dma_start(wtagt[k][:], wtagt_d[128 * k:128 * (k + 1), :])
            nc.vector.memset(hT[:], 0.0)
            nc.vector.memset(cst[:], 0.0)
            nc.vector.memset(hn2[:], 0.0)

            # ---- Phase 1: LSTM recurrence (fully unrolled) ----
            with tc.tile_pool(name="pz", bufs=2, space="PSUM") as pzp, \
                 tc.tile_pool(name="pt", bufs=2, space="PSUM") as ptp:
                def emb_quads(t, pz):
                    for g in range(4):
                        nc.tensor.matmul(pz[32 * g:32 * g + 32, 0:512],
                                         e0[:, 4 * t:4 * t + 32],
                                         strips(wih0, g),
                                         start=True, stop=False,
                                         tile_position=(0, 32 * g),
                                         skip_group_check=True)
                    for g in range(4):
                        nc.tensor.matmul(pz[32 * g:32 * g + 4, 0:512],
                                         e1[:, 4 * t:4 * t + 4],
                                         strips(wih1, g),
                                         start=False, stop=False,
                                         tile_position=(0, 32 * g),
                                         skip_group_check=True)
                    for g in range(4):
                        nc.tensor.matmul(pz[32 * g:32 * g + 4, 0:512],
                                         ones[0:1, 4 * t:4 * t + 4],
                                         strips(wih2, g),
                                         start=False, stop=False,
                                         tile_position=(0, 32 * g),
                                         skip_group_check=True)

                pz_cur = pzp.tile([128, 512], F32, tag="pz", name="pz0")
                emb_quads(0, pz_cur)
                for t in range(steps):
                    pz = pz_cur
                    for k in range(4):
                        for g in range(4):
                            nc.tensor.matmul(pz[32 * g:32 * g + 4, 0:512],
                                             hT[:, 4 * k:4 * k + 4],
                                             strips(whh[k], g),
                                             start=False, stop=(k == 3),
                                             tile_position=(0, 32 * g),
                                             skip_group_check=True)
                    # software pipelining: next step's emb/bias matmuls are
                    # independent of h_t — issue them before the elementwise
                    # so the PE works through them while ACT/DVE run.
                    if t + 1 < steps:
                        pz_cur = pzp.tile([128, 512], F32, tag="pz",
                                          name=f"pz{t + 1}")
                        emb_quads(t + 1, pz_cur)
                    # gates [i|f|o|g]: rows {32g+b} meaningful, full-width ops
                    nc.scalar.activation(sif[:], pz[:, 0:384], SIG)
                    nc.scalar.activation(tg[:], pz[:, 384:512], TANH)
                    nc.vector.tensor_mul(m1[:], sif[:, 128:256], cst[:])
                    nc.vector.tensor_mul(m2[:], sif[:, 0:128], tg[:])
                    nc.vector.tensor_add(cst[:], m1[:], m2[:])
                    nc.scalar.activation(tcl[:], cst[:], TANH)
                    # h to batch-major (4, 512) at partition base 0
                    for g in range(4):
                        nc.vector.tensor_mul(hn2[0:4, 128 * g:128 * (g + 1)],
                                             tcl[32 * g:32 * g + 4, 0:128],
                                             sif[32 * g:32 * g + 4, 256:384])
                    # h -> h.T via 16 tiny col-tiled identity matmuls
                    pt = ptp.tile([128, 16], F32, tag="pt")
                    for g in range(4):
                        for j in range(4):
                            nc.tensor.matmul(
                                pt[32 * j:32 * j + 32, 4 * g:4 * g + 4],
                                hn2[0:4, 128 * g + 32 * j:128 * g + 32 * j + 32],
                                i4s[0:4, 0:4],
                                start=True, stop=True,
                                tile_position=(0, 32 * j))
                    for k in range(4):
                        nc.scalar.copy(hT[:, 4 * k:4 * k + 4],
                                       pt[:, 4 * k:4 * k + 4])
                    nc.scalar.copy(hsT[:, 16 * t:16 * (t + 1)], pt[:])

            # ---- Phase 2: hsum and gGb ----
            hsum = pp.tile([128, 16], F32, tag="hsum")
            gGb = pp.tile([4, 256], F32, tag="gGb")
            nc.vector.tensor_reduce(
                hsum[:], hsT[:].rearrange("p (t kb) -> p kb t", kb=16),
                mybir.AxisListType.X, mybir.AluOpType.add)
            with tc.tile_pool(name="p2", bufs=1, space="PSUM") as p2p:
                pg = p2p.tile([4, 256], F32, tag="pg")
                nc.tensor.matmul(pg[:], sm32[0:1, 0:4], bgt[0:1, 0:256],
                                 start=True, stop=False)
                for k in range(4):
                    nc.tensor.matmul(pg[:], hsum[:, 4 * k:4 * k + 4],
                                     wg1t[k][:], start=False, stop=(k == 3))
                nc.scalar.copy(gGb[:], pg[:])

            # ---- Phase 3: Z.T = tanh(Wg2.T.T @ hs.T + gGb bcast) ----
            zt = pp.tile([128, 2 * sb], BF16, tag="zt")
            hs4 = hsT[:].rearrange("p (t k b) -> p k t b", k=4, b=4)
            with tc.tile_pool(name="p3", bufs=2, space="PSUM") as p3p:
                for m in range(2):
                    for n in range(ntb):
                        pz3 = p3p.tile([128, nw], F32, tag="pz3")
                        nc.tensor.matmul(pz3[:], gGb[0:4, 128 * m:128 * (m + 1)],
                                         indrep[0:4, 0:nw], start=True, stop=False)
                        for k in range(4):
                            rhs = hs4[:, k, tpn * n:tpn * (n + 1), :]
                            nc.tensor.matmul(pz3[:],
                                             wg2t[k][:, 128 * m:128 * (m + 1)],
                                             rhs, start=False, stop=(k == 3))
                        nc.scalar.activation(
                            zt[:, m * sb + nw * n: m * sb + nw * (n + 1)],
                            pz3[:], TANH)

            # ---- Phase 4: tag.T = Wtag.T.T @ Z.T + b_tag ----
            tagT = pp.tile([128, sb], F32, tag="tagT")
            with tc.tile_pool(name="p4", bufs=2, space="PSUM") as p4p:
                for n in range(ntb):
                    pz4 = p4p.tile([20, nw], F32, tag="pz4")
                    nc.tensor.matmul(pz4[:], btag[0:1, 0:20],
                                     ones512[0:1, 0:nw], start=True, stop=False)
                    for k in range(2):
                        nc.tensor.matmul(
                            pz4[:], wtagt[k][:],
                            zt[:, k * sb + nw * n: k * sb + nw * (n + 1)],
                            start=False, stop=(k == 1))
                    nc.scalar.copy(tagT[0:20, nw * n:nw * (n + 1)], pz4[:])

            # ---- Phase 5: log-softmax over batch ----
            etag = pp.tile([128, sb], F32, tag="etag")
            sums = pp.tile([128, steps], F32, tag="sums")
            nc.scalar.activation(etag[0:20, :], tagT[0:20, :],
                                 mybir.ActivationFunctionType.Exp)
            nc.vector.tensor_reduce(
                sums[0:20, :],
                etag[0:20, :].rearrange("p (t b) -> p t b", b=BL),
                mybir.AxisListType.X, mybir.AluOpType.add)
            cc_in = dp.tile([20, steps], F32, tag="cc_in")
            cc_out = dp.tile([20, steps], F32, tag="cc_out")
            nc.gpsimd.dma_start(cc_in[:], sums[0:20, :])
            nc.gpsimd.collective_compute(
                "AllReduce", mybir.AluOpType.add,
                replica_groups=[list(range(NCORES))],
                ins=[cc_in[:].opt()], outs=[cc_out[:].opt()])
            nc.gpsimd.dma_start(sums[0:20, :], cc_out[:])
            nc.scalar.activation(sums[0:20, :], sums[0:20, :],
                                 mybir.ActivationFunctionType.Ln)
            tag3 = tagT[0:20, :].rearrange("p (t b) -> p t b", b=BL)
            for b in range(BL):
                nc.vector.tensor_sub(tag3[:, :, b], tag3[:, :, b],
                                     sums[0:20, :])

            # ---- Phase 6: transpose (20, sb) -> (sb, 20) and write out ----
            obuf = pp.tile([tw, ntr * 20], F32, tag="obuf")
            with tc.tile_pool(name="p6", bufs=2, space="PSUM") as p6p:
                for j in range(ntr):
                    pz6 = p6p.tile([tw, 20], F32, tag="pz6")
                    nc.tensor.transpose(pz6[:],
                                        tagT[0:20, tw * j:tw * (j + 1)],
                                        i20[:])
                    nc.scalar.copy(obuf[:, 20 * j:20 * (j + 1)], pz6[:])
            nc.sync.dma_start(
                out_d[:].rearrange("(j p) k -> p j k", p=tw),
                obuf[0:tw, :].rearrange("p (j k) -> p j k", j=ntr))
    nc.finalize()
    return nc


def _prep_inputs(inputs, steps=S):
    """Host-side prep: gather + transpose + pack per-core shards."""
    x = np.asarray(inputs["x"]).astype(np.int64)[:steps]          # (steps, B)
    embed = np.asarray(inputs["embed"], np.float32)
    W_ih = np.asarray(inputs["W_ih"], np.float32)
    W_hh = np.asarray(inputs["W_hh"], np.float32)
    b_ih = np.asarray(inputs["b_ih"], np.float32)
    b_hh = np.asarray(inputs["b_hh"], np.float32)
    W_g = np.asarray(inputs["W_g"], np.float32)
    b_g = np.asarray(inputs["b_g"], np.float32)
    W_tag = np.asarray(inputs["W_tag"], np.float32)
    b_tag = np.asarray(inputs["b_tag"], np.float32)

    sb = steps * BL
    emb = embed[x]                                                # (steps, B, E)
    wihT = _perm_gates_cols(W_ih.T.astype(np.float32))            # (E, 4H)
    bias = _perm_gates_cols((b_ih + b_hh).astype(np.float32))     # (4H,)

    i4s = np.zeros((128, 4), np.float32)
    i4s[:4, :4] = np.eye(4)
    sm32 = np.zeros((128, 256), np.float32)
    sm32[0, :4] = 1.0
    sm32[32, :256] = b_g
    indrep = np.tile(np.eye(4, dtype=np.float32), (1, 128))
    shared = {
        "ones": np.ones((1, sb), BF16_NP),
        "wih2": bias.reshape(1, 2048).astype(BF16_NP),
        "ones512": np.ones((1, 512), BF16_NP),
        "btag": b_tag.reshape(1, 20).astype(BF16_NP),
        "bgt": b_g.reshape(1, 256).astype(np.float32),
        "wih0": wihT[0:128].astype(BF16_NP),
        "wih1": wihT[128:256].astype(BF16_NP),
        "whh": _perm_gates_cols(W_hh.T.astype(np.float32)).astype(BF16_NP),
        "i4s": i4s,
        "sm32": sm32,
        "wg1t": W_g[:, :H].T.astype(np.float32).copy(),
        "wg2t": W_g[:, H:].T.astype(BF16_NP),
        "indrep": indrep,
        "wtagt": W_tag.T.astype(BF16_NP),
        "i20": np.eye(20, dtype=np.float32),
    }
    in_maps = []
    for c in range(NCORES):
        sl = emb[:, BL * c:BL * (c + 1), :]                       # (steps, BL, E)
        embT = np.ascontiguousarray(sl.transpose(2, 0, 1).reshape(E, sb))
        m = dict(shared)
        e0p = np.zeros((128, sb + 32), np.float32)
        e0p[:, :sb] = embT[0:128]
        m["e0"] = e0p.astype(BF16_NP)
        m["e1"] = embT[128:256].astype(BF16_NP)
        in_maps.append(m)
    return in_maps


def run(inputs, steps=S, trace=False):
    from concourse.bass_utils import run_bass_kernel_spmd
    key = steps
    if key not in _graph_cache:
        _graph_cache[key] = build_graph(steps)
    nc = _graph_cache[key]
    in_maps = _prep_inputs(inputs, steps)
    res = run_bass_kernel_spmd(nc, in_maps, core_ids=list(range(NCORES)),
                               trace=trace)
    outs = [r["out"].reshape(steps, BL, T) for r in res.results]
    full = np.concatenate(outs, axis=1).astype(np.float32)        # (steps, B, T)
    return full, res


def kernel(**inputs):
    out, _ = run(inputs, steps=S, trace=False)
    return out


# revision 13
# speedup vs baseline: 5.4628x; 1.2062x over previous
"""Trainium2 Bass kernel for nn_Att_LSTM_67989332296335.

Math note: the reference's attention softmax is over a singleton axis, so
A == 1 identically and G[t] = sum_j hs[j] for every t — the whole (S,S,B)
distance tensor is dead code.  The live computation is: embedding gather ->
1024-step LSTM -> hsum -> Z = tanh(G @ Wg1.T + hs @ Wg2.T + b_g) ->
tag = Z @ W_tag.T + b_tag -> log_softmax over the batch axis.

Distribution: data-parallel over batch B=32 across 8 cores (B_local=4),
per the sharding hint.  The LSTM recurrence runs per-core with a
column-tiled TensorE schedule: col group g owns hidden-unit chunk
U_g = [128g, 128g+128) x all 4 gates; weights stream as the moving operand
(7 K-chunks of N=512 per group per step: 2 emb + 1 bias-row + 4 W_hh) while
the tiny h.T / emb.T slices are stationary.  h -> h.T each step via 16
small col-tiled identity matmuls.  The batch log-softmax needs one 8-core
AllReduce of the (20, S) exp-sums.
"""
import sys
sys.path.insert(0, "/opt/trn_rl_repo")

import numpy as np
import ml_dtypes

import concourse.bass as bass
import concourse.tile as tile
from concourse import bacc, mybir

F32 = mybir.dt.float32
BF16 = mybir.dt.bfloat16
BF16_NP = ml_dtypes.bfloat16

S, B, E, H, AH, V, T = 1024, 32, 256, 512, 256, 32000, 20
NCORES = 8
BL = B // NCORES  # 4

_graph_cache = {}
GPERM = [0, 1, 3, 2]  # column gate-block order becomes [i, f, o, g]


def _perm_gates_cols(w):
    """Permute the 2048-wide gate axis (last) into [i,f,o,g] block order."""
    shp = w.shape
    v = w.reshape(shp[:-1] + (4, 512))
    return np.ascontiguousarray(v[..., GPERM, :].reshape(shp))


def build_graph(steps=S, debug=False):
    nc = bacc.Bacc(None, target_bir_lowering=False, debug=debug)
    sb = steps * BL
    tpn = min(128, steps)                 # timesteps per P3/P4 N-tile
    nw = tpn * BL                         # N width (cols) per tile
    ntb = steps // tpn                    # number of N-tiles
    tw = min(128, sb)                     # output-transpose tile width
    ntr = sb // tw

    def P(name, shape, dt):
        return nc.dram_tensor(name, list(shape), dt, kind="ExternalInput")

    e0_d = P("e0", (128, sb + 32), BF16)
    e1_d = P("e1", (128, sb), BF16)
    ones_d = P("ones", (1, sb), BF16)
    wih2_d = P("wih2", (1, 2048), BF16)
    ones512_d = P("ones512", (1, 512), BF16)
    btag_d = P("btag", (1, 20), BF16)
    bgt_d = P("bgt", (1, 256), F32)
    wih0_d = P("wih0", (128, 2048), BF16)
    wih1_d = P("wih1", (128, 2048), BF16)
    whh_d = P("whh", (512, 2048), BF16)
    i4s_d = P("i4s", (128, 4), F32)
    sm32_d = P("sm32", (128, 256), F32)
    wg1t_d = P("wg1t", (512, 256), F32)
    wg2t_d = P("wg2t", (512, 256), BF16)
    indrep_d = P("indrep", (4, 512), F32)
    wtagt_d = P("wtagt", (256, 20), BF16)
    i20_d = P("i20", (20, 20), F32)
    out_d = nc.dram_tensor("out", [sb, T], F32, kind="ExternalOutput")

    SIG = mybir.ActivationFunctionType.Sigmoid
    TANH = mybir.ActivationFunctionType.Tanh

    def strips(w, g):
        """Moving-operand AP: cols {512*gate + 128*g + [0:128)} of a (p, 2048) tile."""
        return w[:].rearrange("p (G C u) -> p G C u", G=4, C=4)[:, :, g, :]

    with tile.TileContext(nc) as tc:
        with tc.tile_pool(name="persist", bufs=1) as pp, \
             tc.tile_pool(name="dram", bufs=1, space="DRAM") as dp:
            e0 = pp.tile([128, sb + 32], BF16, tag="e0")
            e1 = pp.tile([128, sb], BF16, tag="e1")
            ones = pp.tile([1, sb], BF16, tag="ones")
            wih2 = pp.tile([1, 2048], BF16, tag="wih2")
            ones512 = pp.tile([1, 512], BF16, tag="ones512")
            btag = pp.tile([1, 20], BF16, tag="btag")
            bgt = pp.tile([1, 256], F32, tag="bgt")
            wih0 = pp.tile([128, 2048], BF16, tag="wih0")
            wih1 = pp.tile([128, 2048], BF16, tag="wih1")
            whh = [pp.tile([128, 2048], BF16, name=f"whh{k}", tag=f"whh{k}") for k in range(4)]
            i4s = pp.tile([128, 4], F32, tag="i4s")
            sm32 = pp.tile([128, 256], F32, tag="sm32")
            wg1t = [pp.tile([128, 256], F32, name=f"wg1t{k}", tag=f"wg1t{k}") for k in range(4)]
            wg2t = [pp.tile([128, 256], BF16, name=f"wg2t{k}", tag=f"wg2t{k}") for k in range(4)]
            indrep = pp.tile([4, 512], F32, tag="indrep")
            wtagt = [pp.tile([128, 20], BF16, name=f"wtagt{k}", tag=f"wtagt{k}") for k in range(2)]
            i20 = pp.tile([20, 20], F32, tag="i20")

            hT = pp.tile([128, 16], BF16, tag="hT")
            hsT = pp.tile([128, steps * 16], BF16, tag="hsT")
            cst = pp.tile([128, 128], F32, tag="cst")
            sif = pp.tile([128, 384], F32, tag="sif")
            tg = pp.tile([128, 128], F32, tag="tg")
            so = pp.tile([128, 128], F32, tag="so")
            tcl = pp.tile([128, 128], F32, tag="tcl")
            m1 = pp.tile([128, 128], F32, tag="m1")
            m2 = pp.tile([128, 128], F32, tag="m2")
            hn2 = pp.tile([4, 512], F32, tag="hn2")

            for t_, src in [(e0, e0_d), (e1, e1_d), (ones, ones_d),
                            (wih2, wih2_d), (ones512, ones512_d),
                            (btag, btag_d), (bgt, bgt_d),
                            (wih0, wih0_d), (wih1, wih1_d), (i4s, i4s_d),
                            (sm32, sm32_d), (indrep, indrep_d), (i20, i20_d)]:
                nc.sync.dma_start(t_[:], src[:])
            for k in range(4):
                nc.sync.dma_start(whh[k][:], whh_d[128 * k:128 * (k + 1), :])
                nc.sync.dma_start(wg1t[k][:], wg1t_d[128 * k:128 * (k + 1), :])
                nc.sync.dma_start(wg2t[k][:], wg2t_d[128 * k:128 * (k + 1), :])
            for k in range(2):
                nc.sync.dma_start(wtagt[k][:], wtagt_d[128 * k:128 * (k + 1), :])
            nc.vector.memset(hT[:], 0.0)
            nc.vector.memset(cst[:], 0.0)
            nc.vector.memset(hn2[:], 0.0)

            # ---- Phase 1: LSTM recurrence (fully unrolled) ----
            with tc.tile_pool(name="pz", bufs=2, space="PSUM") as pzp, \
                 tc.tile_pool(name="pt", bufs=2, space="PSUM") as ptp:
                def emb_quads(t, pz):
                    for g in range(4):
                        nc.tensor.matmul(pz[32 * g:32 * g + 32, 0:512],
                                         e0[:, 4 * t:4 * t + 32],
                                         strips(wih0, g),
                                         start=True, stop=False,
                                         tile_position=(0, 32 * g),
                                         skip_group_check=True)
                    for g in range(4):
                        nc.tensor.matmul(pz[32 * g:32 * g + 4, 0:512],
                                         e1[:, 4 * t:4 * t + 4],
                                         strips(wih1, g),
                                         start=False, stop=False,
                                         tile_position=(0, 32 * g),
                                         skip_group_check=True)
                    for g in range(4):
                        nc.tensor.matmul(pz[32 * g:32 * g + 4, 0:512],
                                         ones[0:1, 4 * t:4 * t + 4],
                                         strips(wih2, g),
                                         start=False, stop=False,
                                         tile_position=(0, 32 * g),
                                         skip_group_check=True)

                pz_cur = pzp.tile([128, 512], F32, tag="pz", name="pz0")
                emb_quads(0, pz_cur)
                for t in range(steps):
                    pz = pz_cur
                    for k in range(4):
                        for g in range(4):
                            nc.tensor.matmul(pz[32 * g:32 * g + 4, 0:512],
                                             hT[:, 4 * k:4 * k + 4],
                                             strips(whh[k], g),
                                             start=False, stop=(k == 3),
                                             tile_position=(0, 32 * g),
                                             skip_group_check=True)
                    # software pipelining: next step's emb/bias matmuls are
                    # independent of h_t — issue them before the elementwise
                    # so the PE works through them while ACT/DVE run.
                    if t + 1 < steps:
                        pz_cur = pzp.tile([128, 512], F32, tag="pz",
                                          name=f"pz{t + 1}")
                        emb_quads(t + 1, pz_cur)
                    # gates [i|f|o|g]: rows {32g+b} meaningful, full-width ops
                    nc.scalar.activation(sif[:], pz[:, 0:384], SIG)
                    nc.scalar.activation(tg[:], pz[:, 384:512], TANH)
                    nc.vector.tensor_mul(m1[:], sif[:, 128:256], cst[:])
                    nc.vector.tensor_mul(m2[:], sif[:, 0:128], tg[:])
                    nc.vector.tensor_add(cst[:], m1[:], m2[:])
                    nc.scalar.activation(tcl[:], cst[:], TANH)
                    # h to batch-major (4, 512) at partition base 0
                    for g in range(4):
                        nc.vector.tensor_mul(hn2[0:4, 128 * g:128 * (g + 1)],
                                             tcl[32 * g:32 * g + 4, 0:128],
                                             sif[32 * g:32 * g + 4, 256:384])
                    # h -> h.T via 16 tiny col-tiled identity matmuls
                    pt = ptp.tile([128, 16], F32, tag="pt")
                    for g in range(4):
                        for j in range(4):
                            nc.tensor.matmul(
                                pt[32 * j:32 * j + 32, 4 * g:4 * g + 4],
                                hn2[0:4, 128 * g + 32 * j:128 * g + 32 * j + 32],
                                i4s[0:4, 0:4],
                                start=True, stop=True,
                                tile_position=(0, 32 * j))
                    nc.vector.tensor_copy(hT[:], pt[:])
                    nc.vector.tensor_copy(hsT[:, 16 * t:16 * (t + 1)], pt[:])

            # ---- Phase 2: hsum and gGb ----
            hsum = pp.tile([128, 16], F32, tag="hsum")
            gGb = pp.tile([4, 256], F32, tag="gGb")
            nc.vector.tensor_reduce(
                hsum[:], hsT[:].rearrange("p (t kb) -> p kb t", kb=16),
                mybir.AxisListType.X, mybir.AluOpType.add)
            with tc.tile_pool(name="p2", bufs=1, space="PSUM") as p2p:
                pg = p2p.tile([4, 256], F32, tag="pg")
                nc.tensor.matmul(pg[:], sm32[0:1, 0:4], bgt[0:1, 0:256],
                                 start=True, stop=False)
                for k in range(4):
                    nc.tensor.matmul(pg[:], hsum[:, 4 * k:4 * k + 4],
                                     wg1t[k][:], start=False, stop=(k == 3))
                nc.scalar.copy(gGb[:], pg[:])

            # ---- Phase 3: Z.T = tanh(Wg2.T.T @ hs.T + gGb bcast) ----
            zt = pp.tile([128, 2 * sb], BF16, tag="zt")
            hs4 = hsT[:].rearrange("p (t k b) -> p k t b", k=4, b=4)
            with tc.tile_pool(name="p3", bufs=2, space="PSUM") as p3p:
                for m in range(2):
                    for n in range(ntb):
                        pz3 = p3p.tile([128, nw], F32, tag="pz3")
                        nc.tensor.matmul(pz3[:], gGb[0:4, 128 * m:128 * (m + 1)],
                                         indrep[0:4, 0:nw], start=True, stop=False)
                        for k in range(4):
                            rhs = hs4[:, k, tpn * n:tpn * (n + 1), :]
                            nc.tensor.matmul(pz3[:],
                                             wg2t[k][:, 128 * m:128 * (m + 1)],
                                             rhs, start=False, stop=(k == 3))
                        nc.scalar.activation(
                            zt[:, m * sb + nw * n: m * sb + nw * (n + 1)],
                            pz3[:], TANH)

            # ---- Phase 4: tag.T = Wtag.T.T @ Z.T + b_tag ----
            tagT = pp.tile([128, sb], F32, tag="tagT")
            with tc.tile_pool(name="p4", bufs=2, space="PSUM") as p4p:
                for n in range(ntb):
                    pz4 = p4p.tile([20, nw], F32, tag="pz4")
                    nc.tensor.matmul(pz4[:], btag[0:1, 0:20],
                                     ones512[0:1, 0:nw], start=True, stop=False)
                    for k in range(2):
                        nc.tensor.matmul(
                            pz4[:], wtagt[k][:],
                            zt[:, k * sb + nw * n: k * sb + nw * (n + 1)],
                            start=False, stop=(k == 1))
                    nc.scalar.copy(tagT[0:20, nw * n:nw * (n + 1)], pz4[:])

            # ---- Phase 5: log-softmax over batch ----
            etag = pp.tile([128, sb], F32, tag="etag")
            sums = pp.tile([128, steps], F32, tag="sums")
            nc.scalar.activation(etag[0:20, :], tagT[0:20, :],
                                 mybir.ActivationFunctionType.Exp)
            nc.vector.tensor_reduce(
                sums[0:20, :],
                etag[0:20, :].rearrange("p (t b) -> p t b", b=BL),
                mybir.AxisListType.X, mybir.AluOpType.add)
            cc_in = dp.tile([20, steps], F32, tag="cc_in")
            cc_out = dp.tile([20, steps], F32, tag="cc_out")
            nc.gpsimd.dma_start(cc_in[:], sums[0:20, :])
            nc.gpsimd.collective_compute(
                "AllReduce", mybir.AluOpType.add,
                replica_groups=[list(range(NCORES))],
                ins=[cc_in[:].opt()], outs=[cc_out[:].opt()])
            nc.gpsimd.dma_start(sums[0:20, :], cc_out[:])
            nc.scalar.activation(sums[0:20, :], sums[0:20, :],
                                 mybir.ActivationFunctionType.Ln)
            tag3 = tagT[0:20, :].rearrange("p (t b) -> p t b", b=BL)
            for b in range(BL):
                nc.vector.tensor_sub(tag3[:, :, b], tag3[:, :, b],
                                     sums[0:20, :])

            # ---- Phase 6: transpose (20, sb) -> (sb, 20) and write out ----
            obuf = pp.tile([tw, ntr * 20], F32, tag="obuf")
            with tc.tile_pool(name="p6", bufs=2, space="PSUM") as p6p:
                for j in range(ntr):
                    pz6 = p6p.tile([tw, 20], F32, tag="pz6")
                    nc.tensor.transpose(pz6[:],
                                        tagT[0:20, tw * j:tw * (j + 1)],
                                        i20[:])
                    nc.scalar.copy(obuf[:, 20 * j:20 * (j + 1)], pz6[:])
            nc.sync.dma_start(
                out_d[:].rearrange("(j p) k -> p j k", p=tw),
                obuf[0:tw, :].rearrange("p (j k) -> p j k", j=ntr))
    nc.finalize()
    return nc


def _prep_inputs(inputs, steps=S):
    """Host-side prep: gather + transpose + pack per-core shards."""
    x = np.asarray(inputs["x"]).astype(np.int64)[:steps]          # (steps, B)
    embed = np.asarray(inputs["embed"], np.float32)
    W_ih = np.asarray(inputs["W_ih"], np.float32)
    W_hh = np.asarray(inputs["W_hh"], np.float32)
    b_ih = np.asarray(inputs["b_ih"], np.float32)
    b_hh = np.asarray(inputs["b_hh"], np.float32)
    W_g = np.asarray(inputs["W_g"], np.float32)
    b_g = np.asarray(inputs["b_g"], np.float32)
    W_tag = np.asarray(inputs["W_tag"], np.float32)
    b_tag = np.asarray(inputs["b_tag"], np.float32)

    sb = steps * BL
    emb = embed[x]                                                # (steps, B, E)
    wihT = _perm_gates_cols(W_ih.T.astype(np.float32))            # (E, 4H)
    bias = _perm_gates_cols((b_ih + b_hh).astype(np.float32))     # (4H,)

    i4s = np.zeros((128, 4), np.float32)
    i4s[:4, :4] = np.eye(4)
    sm32 = np.zeros((128, 256), np.float32)
    sm32[0, :4] = 1.0
    sm32[32, :256] = b_g
    indrep = np.tile(np.eye(4, dtype=np.float32), (1, 128))
    shared = {
        "ones": np.ones((1, sb), BF16_NP),
        "wih2": bias.reshape(1, 2048).astype(BF16_NP),
        "ones512": np.ones((1, 512), BF16_NP),
        "btag": b_tag.reshape(1, 20).astype(BF16_NP),
        "bgt": b_g.reshape(1, 256).astype(np.float32),
        "wih0": wihT[0:128].astype(BF16_NP),
        "wih1": wihT[128:256].astype(BF16_NP),
        "whh": _perm_gates_cols(W_hh.T.astype(np.float32)).astype(BF16_NP),
        "i4s": i4s,
        "sm32": sm32,
        "wg1t": W_g[:, :H].T.astype(np.float32).copy(),
        "wg2t": W_g[:, H:].T.astype(BF16_NP),
        "indrep": indrep,
        "wtagt": W_tag.T.astype(BF16_NP),
        "i20": np.eye(20, dtype=np.float32),
    }
    in_maps = []
    for c in range(NCORES):
        sl = emb[:, BL * c:BL * (c + 1), :]                       # (steps, BL, E)
        embT = np.ascontiguousarray(sl.transpose(2, 0, 1).reshape(E, sb))
        m = dict(shared)
        e0p = np.zeros((128, sb + 32), np.float32)
        e0p[:, :sb] = embT[0:128]
        m["e0"] = e0p.astype(BF16_NP)
        m["e1"] = embT[128:256].astype(BF16_NP)
        in_maps.append(m)
    return in_maps


def run(inputs, steps=S, trace=False):
    from concourse.bass_utils import run_bass_kernel_spmd
    key = steps
    if key not in _graph_cache:
        _graph_cache[key] = build_graph(steps)
    nc = _graph_cache[key]
    in_maps = _prep_inputs(inputs, steps)
    res = run_bass_kernel_spmd(nc, in_maps, core_ids=list(range(NCORES)),
                               trace=trace)
    outs = [r["out"].reshape(steps, BL, T) for r in res.results]
    full = np.concatenate(outs, axis=1).astype(np.float32)        # (steps, B, T)
    return full, res


def kernel(**inputs):
    out, _ = run(inputs, steps=S, trace=False)
    return out
